# revision 6
# baseline (speedup 1.0000x reference)
"""Trainium2 Bass kernel for nn_BiDGNBlock (moe_routing).

Strategy: data-parallel over batch across 8 NeuronCores (no collectives).
Each core computes one batch element end-to-end.

v2 changes over the 56.6us baseline (see work/kernel_baseline_56610.py):
  - Expert masks: ALL 64 experts' partition-replicated masks come from one
    DRAM round-trip (write R.T once, 4 broadcast reads on the idle scalar
    HWDGE queue). The old inline-mask path (top-8 indices, sel matmuls,
    per-group IS_EQ chains) is deleted -- it saturated DVE at stream start,
    delaying asch and dropping the PE clock (HAM) to 4/8 for ~7us.
  - Expert groups widened to EG=8: one DVE mask-multiply per 16 matmuls,
    so DVE stays ~1.4x ahead of PE and LDWEIGHTS pull-ahead works.
  - Input DMA split into 9 need-ordered chunks on two HWDGE queues
    (sync: attention/proj/router weights + weq; scalar: LN vectors + be),
    64-partition tensors shipped as [64, N] (half the bytes of the old
    128-row blobs).  q/k start ~2.5us earlier.
  - softmax exp emitted before the v matmuls (the old order hid the exp
    behind 3 dummy matmuls' semaphore threshold).
  - LN1/LN2 normalize split across engines: DVE does one side from PSUM,
    scalar does the other side's (x-m)*rstd from PSUM (activation with
    per-partition scale/bias), gpsimd does its gamma/beta+residual SBUF
    ops.  gpsimd has no PSUM port, scalar does.
  - Dense dummy-matmul bridges (pinned by data deps) only where the PE
    would otherwise idle >1us: LN1 window and the mask round-trip window.

Precision: attention/router matmuls in fp32 LOW_HIGH (2-pass) -- the
top-2 routing margin is 4.4e-5, fp16/fp22 provably flips a pick.  We
table We as fp8e4 scaled x128 (final layer_norm absorbs the scale),
expert activations fp16.
"""

import sys
import numpy as np

sys.path.insert(0, "/opt/trn_rl_repo")

N_CORES = 8
B, C, T = 8, 64, 256
EXP = 32
KT = T // 128  # 2 k-tiles over the feature dim
WE_SCALE = 128.0
EG = 8         # experts per asch group

_CACHE: dict = {}

# chunk specs: name -> (partitions, [(field, shape), ...])
# shape[0] is the field's partition count; cols = prod(shape[1:]).
CHUNK_SPECS = {
    "cA": (128, [("xtl", (128, KT, C)), ("xtr", (128, KT, C)),
                 ("wqt", (128, KT, T)), ("bqp", (128, KT)),
                 ("bkp", (128, KT))]),
    "cK": (128, [("wkt", (128, KT, T))]),
    "cV": (128, [("wvt", (128, KT, T))]),
    "cVb": (64, [("bvr", (64, T)), ("bprr", (64, 2 * T)),
                 ("ident", (64, 64))]),
    "cP": (128, [("wpt", (128, KT, T))]),
    "cR": (128, [("wrt", (128, 2 * KT, EXP)), ("mgboth", (128, T))]),
    "cRb": (64, [("xlb", (64, T)), ("xrb", (64, T)),
                 ("aglr", (64, T)), ("agrr", (64, T)),
                 ("obbl", (64, T)), ("obb2", (64, T)),
                 ("cent", (32, C)), ("brp", (32, 1))]),
}


def _layouts():
    out = {}
    for cname, (parts, fields) in CHUNK_SPECS.items():
        off = 0
        for name, shape in fields:
            cols = int(np.prod(shape[1:]))
            out[name] = (cname, off, shape)
            off += cols
        out[cname + "__ncols"] = (cname, off, None)
    return out


FIELD_OFF = _layouts()


def _chunk_cols(cname):
    return FIELD_OFF[cname + "__ncols"][1]


def _build():
    import concourse.bass as bass
    import concourse.mybir as mybir
    import concourse.tile as tile
    from concourse import bacc
    from contextlib import ExitStack

    dt = mybir.dt
    f32, f16, f8 = dt.float32, dt.float16, dt.float8e4
    AF = mybir.ActivationFunctionType
    OP = mybir.AluOpType

    nc = bacc.Bacc("TRN2", target_bir_lowering=False, debug=False,
                   num_devices=N_CORES)

    dram_in = {}
    for cname, (parts, fields) in CHUNK_SPECS.items():
        dram_in[cname] = nc.dram_tensor(cname, [parts, _chunk_cols(cname)],
                                        f32, kind="ExternalInput")
    behs_d = nc.dram_tensor("behs", [C, T], f16, kind="ExternalInput")
    weq_d = nc.dram_tensor("weq", [128, C, KT, T], f8, kind="ExternalInput")
    ob_d = nc.dram_tensor("oboth", [128, T], f32, kind="ExternalOutput")

    with tile.TileContext(nc) as tc, ExitStack() as ctx:
        cst = ctx.enter_context(tc.tile_pool(name="cst", bufs=1))
        wk = ctx.enter_context(tc.tile_pool(name="wk", bufs=2))
        sm = ctx.enter_context(tc.tile_pool(name="sm", bufs=2))
        asc_p = ctx.enter_context(tc.tile_pool(name="asc", bufs=3))
        ps = ctx.enter_context(tc.tile_pool(name="ps", bufs=2, space="PSUM"))
        proj_p = ctx.enter_context(tc.tile_pool(name="proj", bufs=1, space="PSUM"))
        moe_p = ctx.enter_context(tc.tile_pool(name="moe", bufs=1, space="PSUM"))
        warm_p = ctx.enter_context(tc.tile_pool(name="warm", bufs=1, space="PSUM"))
        dram = ctx.enter_context(tc.tile_pool(name="dram", bufs=1, space="DRAM"))

        # ---- input DMAs, need-ordered.  sync queue: critical-path weights
        # then the big expert table; scalar queue: LN vectors + be (so the
        # mask round-trip later finds an empty queue is NOT needed -- the
        # round-trip rides sync's second queue as in the baseline). ----
        sb = {}
        for cname in ("cA", "cK", "cV", "cVb", "cP", "cR"):
            parts = CHUNK_SPECS[cname][0]
            t_ = cst.tile([parts, _chunk_cols(cname)], f32, tag=cname)
            nc.sync.dma_start(out=t_, in_=dram_in[cname].ap())
            sb[cname] = t_
        cRb_t = cst.tile([64, _chunk_cols("cRb")], f32, tag="cRb")
        nc.scalar.dma_start(out=cRb_t, in_=dram_in["cRb"].ap())
        sb["cRb"] = cRb_t
        behs = cst.tile([C, T], f16, tag="behs")
        nc.scalar.dma_start(out=behs, in_=behs_d.ap())
        weq = cst.tile([128, C, KT, T], f8, tag="weq")
        wea = weq_d.ap()
        for ch in range(4):
            nc.sync.dma_start(out=weq[:, ch * 16:(ch + 1) * 16],
                              in_=wea[:, ch * 16:(ch + 1) * 16])

        def fview(name):
            cname, off, shape = FIELD_OFF[name]
            parts = shape[0]
            cols = int(np.prod(shape[1:]))
            v = sb[cname][0:parts, off:off + cols]
            if len(shape) == 3:
                v = v.rearrange("p (a b) -> p a b", a=shape[1])
            return v

        xtl = fview("xtl")
        xtr = fview("xtr")
        wqt = fview("wqt")
        bqp = fview("bqp")
        bkp = fview("bkp")
        wkt = fview("wkt")
        wvt = fview("wvt")
        bvr = fview("bvr")
        bprr = fview("bprr")
        ident = fview("ident")
        wpt = fview("wpt")
        wrt = fview("wrt")
        mgboth = fview("mgboth")
        xlb = fview("xlb")
        xrb = fview("xrb")
        aglr = fview("aglr")
        agrr = fview("agrr")
        obbl = fview("obbl")
        obb2 = fview("obb2")
        cent = fview("cent")
        brp = fview("brp")

        # ---- PE warm-up from memset tiles + ACT table preloads ----
        wsrc = cst.tile([128, 512], f16, tag="wsrc")
        nc.vector.memset(wsrc, 0.5)
        pw = warm_p.tile([128, 512], f32, tag="warm")
        for wi in range(4):
            nc.tensor.matmul(pw, wsrc[:, 0:128], wsrc,
                             start=True, stop=True, skip_group_check=True)
        wact = cst.tile([1, 32], f32, tag="wact")
        nc.vector.memset(wact, 1.0)
        nc.scalar.activation(out=wact, in_=wact, func=AF.Sqrt)
        nc.scalar.activation(out=wact, in_=wact, func=AF.Exp)

        wsrc32 = cst.tile([128, 512], f32, tag="wsrc32")
        nc.vector.memset(wsrc32, 0.25)

        def dmy(dep, n=1, cols=96):
            # keep-warm matmul pinned in time by a data dependency: the
            # scheduler cannot hoist it ahead of `dep` (SBUF tile).
            kp = dep.shape[0]
            lhs = dep if dep.shape[-1] <= 64 else dep[:, 0:64]
            mov = wsrc if dep.dtype == f16 else wsrc32
            for _ in range(n):
                nc.tensor.matmul(pw[0:64, 0:cols],
                                 lhs, mov[0:kp, 0:cols],
                                 start=True, stop=True, skip_group_check=True)

        # identh for the fp16 mask transpose (gpsimd, idle early)
        identh = cst.tile([64, 64], f16, tag="identh")
        nc.gpsimd.tensor_copy(identh, ident)

        eps_t = cst.tile([C, 1], f32, tag="eps")
        nc.vector.memset(eps_t, 1e-5)
        eps128 = cst.tile([128, 1], f32, tag="eps128")
        nc.vector.memset(eps128, 1e-5)

        # ---- attention: q.T, k.T ----
        qt = wk.tile([128, KT, C], f32, tag="qt")
        ktl = wk.tile([128, KT, C], f32, tag="ktl")
        for (src, w, bias, dst) in [(xtl, wqt, bqp, qt), (xtr, wkt, bkp, ktl)]:
            for ut in range(KT):
                p = ps.tile([128, C], f32, tag="ps")
                for kt in range(KT):
                    nc.tensor.matmul(p, w[:, kt, ut * 128:(ut + 1) * 128],
                                     src[:, kt], start=(kt == 0), stop=(kt == KT - 1))
                nc.vector.tensor_scalar(out=dst[:, ut], in0=p,
                                        scalar1=bias[:, ut:ut + 1],
                                        scalar2=None, op0=OP.add)

        # ---- energy -> exp(energy/16), rowsum fused via accum_out ----
        # (|energy/16| < 0.75 so no row-max subtraction is needed)
        pe_ = ps.tile([C, C], f32, tag="ps")
        for ut in range(KT):
            nc.tensor.matmul(pe_, qt[:, ut], ktl[:, ut],
                             start=(ut == 0), stop=(ut == KT - 1))
        attn = wk.tile([C, C], f32, tag="attn")
        rowsum = sm.tile([C, 1], f32, tag="rowsum")
        nc.scalar.activation(out=attn, in_=pe_, func=AF.Exp, scale=1.0 / 16.0,
                             accum_out=rowsum)
        nc.vector.reciprocal(rowsum, rowsum)

        # ---- v = (x_l - x_r) @ Wv.T + bv  (fills the softmax PE window) ----
        xdt = wk.tile([128, KT, C], f32, tag="xdt")
        nc.vector.tensor_sub(xdt, xtl, xtr)
        pv = ps.tile([C, T], f32, tag="ps")
        for kt in range(KT):
            nc.tensor.matmul(pv, xdt[:, kt], wvt[:, kt],
                             start=(kt == 0), stop=(kt == KT - 1))
        v_sb = wk.tile([C, T], f32, tag="v")
        nc.vector.tensor_tensor(out=v_sb, in0=pv, in1=bvr, op=OP.add)
        nc.vector.tensor_scalar_mul(attn, attn, rowsum)

        # ---- attn.T ----
        pat = ps.tile([C, C], f32, tag="ps")
        nc.tensor.transpose(pat, attn, ident)
        attnT = wk.tile([C, C], f32, tag="attnT")
        nc.vector.tensor_copy(attnT, pat)

        # ---- proj psum bias pre-init; both sides share one PSUM tile so
        # LN1 stats run fused ----
        ppb = proj_p.tile([C, 2, T], f32, tag="projb")
        nc.vector.tensor_copy(ppb, bprr)

        # ---- out_l.T / out_r.T  [u, c] ----
        oLT = wk.tile([128, KT, C], f32, tag="oLT")
        oRT = wk.tile([128, KT, C], f32, tag="oRT")
        for ut in range(KT):
            pl = ps.tile([128, C], f32, tag="ps")
            nc.tensor.matmul(pl, v_sb[:, ut * 128:(ut + 1) * 128], attnT,
                             start=True, stop=True)
            nc.vector.tensor_copy(oLT[:, ut], pl)
            pr = ps.tile([128, C], f32, tag="ps")
            nc.tensor.matmul(pr, v_sb[:, ut * 128:(ut + 1) * 128], attn,
                             start=True, stop=True)
            nc.scalar.copy(oRT[:, ut], pr)

        # ---- proj (accumulates onto pre-loaded bias) ----
        for ut in range(KT):
            nc.tensor.matmul(ppb[:, 0], oLT[:, ut], wpt[:, ut],
                             start=False, stop=(ut == KT - 1))
        for ut in range(KT):
            nc.tensor.matmul(ppb[:, 1], oRT[:, ut], wpt[:, ut],
                             start=False, stop=(ut == KT - 1))
        # dense PE bridge over the LN1 window (HAM stays at 8/8)
        dmy(oLT[:, 0], 5, 512)

        # ---- LN1: stats fused over the shared [C,2,T] PSUM tile; the
        # normalize is split: side L on DVE, side R via scalar (PSUM
        # (x-m)*rstd) + gpsimd (gamma, beta+residual). ----
        OUT_L = wk.tile([C, T], f32, tag="OUTL")
        OUT_R = wk.tile([C, T], f32, tag="OUTR")
        st1 = sm.tile([C, 2, 6], f32, tag="st1")
        nc.vector.bn_stats(out=st1[:, 0], in_=ppb[:, 0])
        nc.vector.bn_stats(out=st1[:, 1], in_=ppb[:, 1])
        mv1 = sm.tile([C, 2, 2], f32, tag="mv1")
        nc.vector.bn_aggr(out=mv1[:, 0], in_=st1[:, 0])
        nc.vector.bn_aggr(out=mv1[:, 1], in_=st1[:, 1])
        rstd1 = sm.tile([C, 2], f32, tag="rs1")
        nc.scalar.activation(out=rstd1, in_=mv1[:, :, 1:2], func=AF.Sqrt,
                             bias=eps_t)
        nc.vector.reciprocal(rstd1, rstd1)
        # negmr[c,s] = -mean*rstd (bias for the scalar-engine normalize)
        negmr = sm.tile([C, 2], f32, tag="negmr")
        nc.vector.tensor_tensor(out=negmr, in0=mv1[:, :, 0], in1=rstd1,
                                op=OP.mult)
        nc.vector.tensor_scalar(out=negmr, in0=negmr, scalar1=-1.0,
                                scalar2=None, op0=OP.mult)
        # side R: scalar reads PSUM, gpsimd applies gamma and beta+residual
        nc.scalar.activation(out=OUT_R, in_=ppb[:, 1], func=AF.Identity,
                             bias=negmr[:, 1:2], scale=rstd1[:, 1:2])
        nc.gpsimd.tensor_tensor(out=OUT_R, in0=OUT_R, in1=agrr, op=OP.mult)
        nc.gpsimd.tensor_tensor(out=OUT_R, in0=OUT_R, in1=xrb, op=OP.add)
        # side L: DVE
        nc.vector.tensor_scalar(out=OUT_L, in0=ppb[:, 0],
                                scalar1=mv1[:, 0, 0:1],
                                scalar2=rstd1[:, 0:1],
                                op0=OP.subtract, op1=OP.mult)
        nc.vector.tensor_tensor(out=OUT_L, in0=OUT_L, in1=aglr, op=OP.mult)
        nc.vector.tensor_tensor(out=OUT_L, in0=OUT_L, in1=xlb, op=OP.add)

        # ---- transposes -> [u(128), kt, c] f32 (router) + f16 oAll (experts) ----
        oLT2 = wk.tile([128, KT, C], f32, tag="oLT2")
        oRT2 = wk.tile([128, KT, C], f32, tag="oRT2")
        oAll = wk.tile([128, KT, 2, C], f16, tag="oAll")
        for (side, src, dst) in [(0, OUT_L, oLT2), (1, OUT_R, oRT2)]:
            for ut in range(KT):
                pt = ps.tile([128, C], f32, tag="ps")
                nc.tensor.transpose(pt, src[:, ut * 128:(ut + 1) * 128], ident)
                nc.vector.tensor_copy(dst[:, ut], pt)
                nc.scalar.copy(oAll[:, ut, side], pt)

        # ---- router: xp.T then sims (no norms: top-k is scale-invariant
        #      per row and the top-2 softmax gate is exactly 1.0) ----
        rtiles = [(oLT2, 0), (oLT2, 1), (oRT2, 0), (oRT2, 1)]
        pxp = ps.tile([EXP, C], f32, tag="ps")
        for j, (tl, kt) in enumerate(rtiles):
            nc.tensor.matmul(pxp, wrt[:, j], tl[:, kt],
                             start=(j == 0), stop=(j == 3))
        xpT = sm.tile([EXP, C], f32, tag="xpT")
        nc.vector.tensor_scalar(out=xpT, in0=pxp, scalar1=brp, scalar2=None,
                                op0=OP.add)
        psim = ps.tile([C, C], f32, tag="ps")
        nc.tensor.matmul(psim, xpT, cent, start=True, stop=True)

        # top-8 straight off PSUM; full routing matrix via threshold:
        # R[c, e] = (sim >= 2nd-largest).  No exact ties on this input.
        mx8 = sm.tile([C, 8], f32, tag="mx8")
        nc.vector.max(out=mx8, in_=psim)
        maskt = sm.tile([C, C], f16, tag="maskt")
        nc.vector.tensor_scalar(out=maskt, in0=psim, scalar1=mx8[:, 1:2],
                                scalar2=None, op0=OP.is_ge)
        pmt = ps.tile([C, C], f16, tag="psh")
        nc.tensor.transpose(pmt, maskt, identh)
        RTeh = wk.tile([C, C], f16, tag="RTeh")
        nc.vector.tensor_copy(RTeh, pmt)
        # ALL 64 experts' replicated masks via DRAM round-trip on the
        # scalar HWDGE queue (write + 4 pipelined broadcast reads)
        rtd = dram.tile([C, C], f16)
        nc.scalar.dma_start(out=rtd[:], in_=RTeh)
        rsrc = rtd[:]
        mreps = []
        for rc in range(4):
            mt = wk.tile([128, 16, C], f16, tag=f"mrep{rc}")
            src_ap = bass.AP(tensor=rsrc.tensor,
                             offset=rsrc.offset + rc * 16 * C,
                             ap=[[0, 128], [C, 16], [1, C]])
            nc.scalar.dma_start(out=mt, in_=src_ap)
            mreps.append(mt)
        # dense PE bridge over the mask round-trip window
        dmy(RTeh, 8, 512)

        # ---- expert stage (e0 starts the PSUM accumulation group) ----
        ps_moe = moe_p.tile([128, T], f32, tag="psmoe")

        def asch_mult(dst, msrc_ap):
            # dst[p, e(EG), kt, side, c] = oAll[p, kt, side, c] * m[e, c]
            out_ap = bass.AP(tensor=dst.tensor, offset=dst.offset,
                             ap=[list(dst.ap[0]), [KT * 2 * C, EG],
                                 [1, KT * 2 * C]])
            in0 = bass.AP(tensor=oAll.tensor, offset=oAll.offset,
                          ap=[list(oAll.ap[0]), [0, EG], [1, KT * 2 * C]])
            nc.vector.tensor_tensor(out=out_ap, in0=in0, in1=msrc_ap, op=OP.mult)

        def mask_bcast_ap(mt, col0):
            # [p, e(EG), kt*side(bcast), c] view of a [128, 16, C] tile
            return bass.AP(tensor=mt.tensor, offset=mt.offset + col0 * C,
                           ap=[list(mt.ap[0]), [C, EG], [0, KT * 2], [1, C]])

        NG = C // EG
        for g in range(NG):
            e0 = g * EG
            msrc = mask_bcast_ap(mreps[e0 // 16], e0 % 16)
            asch = asc_p.tile([128, EG, KT, 2, C], f16, tag="asc")
            asch_mult(asch, msrc)
            for i in range(EG):
                for kt in range(KT):
                    nc.tensor.matmul(
                        ps_moe, asch[:, i, kt], weq[:, e0 + i, kt],
                        start=(g == 0 and i == 0 and kt == 0),
                        stop=(g == NG - 1 and i == EG - 1
                              and kt == KT - 1),
                        skip_group_check=True)
            if g == 3:
                # bias (be * WE_SCALE) via R matmuls, off the LN2 path and
                # inside the k=8/8 zone
                nc.tensor.matmul(ps_moe[0:C], RTeh, behs, start=False,
                                 stop=False, skip_group_check=True)
                nc.tensor.matmul(ps_moe[C:128], RTeh, behs, start=False,
                                 stop=False, skip_group_check=True)

        # ---- beta+residual for LN2 on gpsimd during the expert phase;
        # the R side is partition-shifted to rows 64:128 by an on-chip
        # SBUF->SBUF DMA (engines need aligned start partitions) ----
        obboth = wk.tile([128, T], f32, tag="obboth")
        obr = wk.tile([C, T], f32, tag="obr")
        nc.gpsimd.tensor_add(obboth[0:C], OUT_L, obbl)
        nc.gpsimd.tensor_add(obr, OUT_R, obb2)
        nc.gpsimd.dma_start(out=obboth[C:128], in_=obr)

        # ---- final LN + residual: both sides live on disjoint partition
        # ranges of ps_moe, so ONE fused stats chain covers them; the
        # normalize splits rows 0:64 (DVE) / 64:128 (scalar+gpsimd) ----
        obv = wk.tile([128, T], f32, tag="obv")
        st2 = sm.tile([128, 6], f32, tag="st2")
        nc.vector.bn_stats(out=st2, in_=ps_moe)
        mv2 = sm.tile([128, 2], f32, tag="mv2")
        nc.vector.bn_aggr(out=mv2, in_=st2)
        rstd2 = sm.tile([128, 1], f32, tag="rs2")
        nc.scalar.activation(out=rstd2, in_=mv2[:, 1:2], func=AF.Sqrt,
                             bias=eps128)
        nc.vector.reciprocal(rstd2, rstd2)
        negmr2 = sm.tile([128, 1], f32, tag="negmr2")
        nc.vector.tensor_tensor(out=negmr2, in0=mv2[:, 0:1], in1=rstd2,
                                op=OP.mult)
        nc.vector.tensor_scalar(out=negmr2, in0=negmr2, scalar1=-1.0,
                                scalar2=None, op0=OP.mult)
        # rows 64:128 via scalar (PSUM) + gpsimd (SBUF)
        nc.scalar.activation(out=obv[C:128], in_=ps_moe[C:128], func=AF.Identity,
                             bias=negmr2[C:128], scale=rstd2[C:128])
        nc.gpsimd.tensor_tensor(out=obv[C:128], in0=obv[C:128],
                                in1=mgboth[C:128], op=OP.mult)
        nc.gpsimd.tensor_tensor(out=obv[C:128], in0=obv[C:128],
                                in1=obboth[C:128], op=OP.add)
        # rows 0:64 on DVE
        nc.vector.tensor_scalar(out=obv[0:C], in0=ps_moe[0:C],
                                scalar1=mv2[0:C, 0:1], scalar2=rstd2[0:C],
                                op0=OP.subtract, op1=OP.mult)
        nc.vector.tensor_tensor(out=obv[0:C], in0=obv[0:C], in1=mgboth[0:C],
                                op=OP.mult)
        nc.vector.tensor_tensor(out=obv[0:C], in0=obv[0:C], in1=obboth[0:C],
                                op=OP.add)
        nc.scalar.dma_start(out=ob_d.ap(), in_=obv)

    nc.compile()
    return nc


def _tile_t(w):
    # (T_in, N) -> [128, T_in//128, N] partition-tiled
    t_in, n = w.shape
    return np.ascontiguousarray(w.reshape(t_in // 128, 128, n).transpose(1, 0, 2))


def _prep_in_maps(inputs):
    import ml_dtypes

    f = np.float32
    x_l, x_r = inputs["x_l"], inputs["x_r"]

    def rep(v):
        return np.repeat(np.asarray(v, f).reshape(1, T), C, axis=0)

    cen = np.asarray(inputs["centers"], f)
    cenn = cen / np.maximum(np.linalg.norm(cen, axis=-1, keepdims=True), 1e-12)
    arrs = {
        "wqt": _tile_t(np.asarray(inputs["Wq"], f).T),
        "wkt": _tile_t(np.asarray(inputs["Wk"], f).T),
        "wvt": _tile_t(np.asarray(inputs["Wv"], f).T),
        "wpt": _tile_t(np.asarray(inputs["Wp"], f).T),
        "bqp": np.asarray(inputs["bq"], f).reshape(KT, 128).T,
        "bkp": np.asarray(inputs["bk"], f).reshape(KT, 128).T,
        "wrt": _tile_t(np.asarray(inputs["Wr"], f).T),
        "brp": np.asarray(inputs["br"], f).reshape(EXP, 1),
        "cent": np.ascontiguousarray(cenn.T),
        "ident": np.eye(64, dtype=f),
        "bvr": rep(inputs["bv"]),
        "bprr": np.concatenate([rep(inputs["bp"]), rep(inputs["bp"])], axis=1),
        "aglr": rep(inputs["ag_l"]), "agrr": rep(inputs["ag_r"]),
        "mgboth": np.concatenate([rep(inputs["mg_l"]), rep(inputs["mg_r"])]),
        "obbl": rep(inputs["mb_l"]), "obb2": rep(inputs["mb_r"]),
        "xlb": np.zeros((C, T), f), "xrb": np.zeros((C, T), f),
    }
    behs = (np.asarray(inputs["be"], f) * WE_SCALE).astype(np.float16)
    # We -> [128(t%128), C, KT, T(u)] fp8e4, x128 (layer_norm absorbs it)
    We = np.asarray(inputs["We"], f)
    WeT = We.transpose(0, 2, 1).reshape(C, KT, 128, T).transpose(2, 0, 1, 3)
    weq = np.ascontiguousarray(WeT * WE_SCALE).astype(ml_dtypes.float8_e4m3)

    def pack(cname, extra):
        parts = CHUNK_SPECS[cname][0]
        blob = np.zeros((parts, _chunk_cols(cname)), f)
        for name, shape in CHUNK_SPECS[cname][1]:
            _, off, _ = FIELD_OFF[name]
            cols = int(np.prod(shape[1:]))
            a = extra[name] if name in extra else arrs[name]
            blob[0:shape[0], off:off + cols] = \
                np.asarray(a, f).reshape(shape[0], cols)
        return blob

    static = {c: pack(c, {}) for c in
              ("cK", "cV", "cVb", "cP", "cR")}
    abl = np.asarray(inputs["ab_l"], f).reshape(1, T)
    abr = np.asarray(inputs["ab_r"], f).reshape(1, T)
    in_maps = []
    for b in range(N_CORES):
        xtl = _tile_t(np.ascontiguousarray(np.asarray(x_l[b], f).T))
        xtr = _tile_t(np.ascontiguousarray(np.asarray(x_r[b], f).T))
        cA = pack("cA", {"xtl": xtl, "xtr": xtr})
        cRb = pack("cRb", {"xlb": np.asarray(x_l[b], f) + abl,
                           "xrb": np.asarray(x_r[b], f) + abr})
        m = {"cA": cA, "cRb": cRb, "behs": behs, "weq": weq}
        m.update(static)
        in_maps.append(m)
    return in_maps


def kernel(**inputs) -> np.ndarray:
    from concourse.bass_utils import run_bass_kernel_spmd

    if "nc" not in _CACHE:
        _CACHE["nc"] = _build()
    nc = _CACHE["nc"]
    in_maps = _prep_in_maps(inputs)
    res = run_bass_kernel_spmd(nc, in_maps, list(range(N_CORES)))
    _CACHE["exec_time_ns"] = res.exec_time_ns
    both = np.stack([res.results[b]["oboth"] for b in range(N_CORES)])
    return np.stack([both[:, 0:C], both[:, C:128]]).astype(np.float32)


# revision 12
# speedup vs baseline: 1.0324x; 1.0324x over previous
"""Trainium2 Bass kernel for nn_BiDGNBlock (moe_routing).

Strategy: data-parallel over batch across 8 NeuronCores (no collectives).
Each core computes one batch element end-to-end.

v2 changes over the 56.6us baseline (see work/kernel_baseline_56610.py):
  - Expert masks: ALL 64 experts' partition-replicated masks come from one
    DRAM round-trip (write R.T once, 4 broadcast reads on the idle scalar
    HWDGE queue). The old inline-mask path (top-8 indices, sel matmuls,
    per-group IS_EQ chains) is deleted -- it saturated DVE at stream start,
    delaying asch and dropping the PE clock (HAM) to 4/8 for ~7us.
  - Expert groups widened to EG=8: one DVE mask-multiply per 16 matmuls,
    so DVE stays ~1.4x ahead of PE and LDWEIGHTS pull-ahead works.
  - Input DMA split into 9 need-ordered chunks on two HWDGE queues
    (sync: attention/proj/router weights + weq; scalar: LN vectors + be),
    64-partition tensors shipped as [64, N] (half the bytes of the old
    128-row blobs).  q/k start ~2.5us earlier.
  - softmax exp emitted before the v matmuls (the old order hid the exp
    behind 3 dummy matmuls' semaphore threshold).
  - LN1/LN2 normalize split across engines: DVE does one side from PSUM,
    scalar does the other side's (x-m)*rstd from PSUM (activation with
    per-partition scale/bias), gpsimd does its gamma/beta+residual SBUF
    ops.  gpsimd has no PSUM port, scalar does.
  - Dense dummy-matmul bridges (pinned by data deps) only where the PE
    would otherwise idle >1us: LN1 window and the mask round-trip window.

Precision: attention/router matmuls in fp32 LOW_HIGH (2-pass) -- the
top-2 routing margin is 4.4e-5, fp16/fp22 provably flips a pick.  We
table We as fp8e4 scaled x128 (final layer_norm absorbs the scale),
expert activations fp16.
"""

import sys
import numpy as np

sys.path.insert(0, "/opt/trn_rl_repo")

N_CORES = 8
B, C, T = 8, 64, 256
EXP = 32
KT = T // 128  # 2 k-tiles over the feature dim
WE_SCALE = 128.0
EG = 8         # experts per asch group

_CACHE: dict = {}

# chunk specs: name -> (partitions, [(field, shape), ...])
# shape[0] is the field's partition count; cols = prod(shape[1:]).
CHUNK_SPECS = {
    "cA": (128, [("xtl", (128, KT, C)), ("xtr", (128, KT, C)),
                 ("wqt", (128, KT, T)), ("bqp", (128, KT)),
                 ("bkp", (128, KT))]),
    "cK": (128, [("wkt", (128, KT, T))]),
    "cV": (128, [("wvt", (128, KT, T))]),
    "cVb": (64, [("bvr", (64, T)), ("bprr", (64, 2 * T)),
                 ("ident", (64, 64))]),
    "cP": (128, [("wpt", (128, KT, T))]),
    "cR": (128, [("wrt", (128, 2 * KT, EXP)), ("mgboth", (128, T))]),
    "cRb": (64, [("xlb", (64, T)), ("xrb", (64, T)),
                 ("aglr", (64, T)), ("agrr", (64, T)),
                 ("obbl", (64, T)), ("obb2", (64, T)),
                 ("cent", (32, C)), ("brp", (32, 1))]),
}


def _layouts():
    out = {}
    for cname, (parts, fields) in CHUNK_SPECS.items():
        off = 0
        for name, shape in fields:
            cols = int(np.prod(shape[1:]))
            out[name] = (cname, off, shape)
            off += cols
        out[cname + "__ncols"] = (cname, off, None)
    return out


FIELD_OFF = _layouts()


def _chunk_cols(cname):
    return FIELD_OFF[cname + "__ncols"][1]


def _build():
    import concourse.bass as bass
    import concourse.mybir as mybir
    import concourse.tile as tile
    from concourse import bacc
    from contextlib import ExitStack

    dt = mybir.dt
    f32, f16, f8 = dt.float32, dt.float16, dt.float8e4
    AF = mybir.ActivationFunctionType
    OP = mybir.AluOpType

    nc = bacc.Bacc("TRN2", target_bir_lowering=False, debug=False,
                   num_devices=N_CORES)

    dram_in = {}
    for cname, (parts, fields) in CHUNK_SPECS.items():
        dram_in[cname] = nc.dram_tensor(cname, [parts, _chunk_cols(cname)],
                                        f32, kind="ExternalInput")
    behs_d = nc.dram_tensor("behs", [C, T], f16, kind="ExternalInput")
    weq_d = nc.dram_tensor("weq", [128, C, KT, T], f8, kind="ExternalInput")
    ob_d = nc.dram_tensor("oboth", [128, T], f32, kind="ExternalOutput")

    with tile.TileContext(nc) as tc, ExitStack() as ctx:
        cst = ctx.enter_context(tc.tile_pool(name="cst", bufs=1))
        wk = ctx.enter_context(tc.tile_pool(name="wk", bufs=2))
        sm = ctx.enter_context(tc.tile_pool(name="sm", bufs=2))
        asc_p = ctx.enter_context(tc.tile_pool(name="asc", bufs=3))
        ps = ctx.enter_context(tc.tile_pool(name="ps", bufs=2, space="PSUM"))
        proj_p = ctx.enter_context(tc.tile_pool(name="proj", bufs=1, space="PSUM"))
        moe_p = ctx.enter_context(tc.tile_pool(name="moe", bufs=1, space="PSUM"))
        warm_p = ctx.enter_context(tc.tile_pool(name="warm", bufs=1, space="PSUM"))
        dram = ctx.enter_context(tc.tile_pool(name="dram", bufs=1, space="DRAM"))

        # ---- input DMAs, need-ordered.  sync queue: critical-path weights
        # then the big expert table; scalar queue: LN vectors + be (so the
        # mask round-trip later finds an empty queue is NOT needed -- the
        # round-trip rides sync's second queue as in the baseline). ----
        sb = {}
        for cname in ("cA", "cK", "cV", "cVb", "cP", "cR"):
            parts = CHUNK_SPECS[cname][0]
            t_ = cst.tile([parts, _chunk_cols(cname)], f32, tag=cname)
            nc.sync.dma_start(out=t_, in_=dram_in[cname].ap())
            sb[cname] = t_
        cRb_t = cst.tile([64, _chunk_cols("cRb")], f32, tag="cRb")
        nc.scalar.dma_start(out=cRb_t, in_=dram_in["cRb"].ap())
        sb["cRb"] = cRb_t
        behs = cst.tile([C, T], f16, tag="behs")
        nc.scalar.dma_start(out=behs, in_=behs_d.ap())
        weq = cst.tile([128, C, KT, T], f8, tag="weq")
        wea = weq_d.ap()
        for ch in range(4):
            nc.sync.dma_start(out=weq[:, ch * 16:(ch + 1) * 16],
                              in_=wea[:, ch * 16:(ch + 1) * 16])

        def fview(name):
            cname, off, shape = FIELD_OFF[name]
            parts = shape[0]
            cols = int(np.prod(shape[1:]))
            v = sb[cname][0:parts, off:off + cols]
            if len(shape) == 3:
                v = v.rearrange("p (a b) -> p a b", a=shape[1])
            return v

        xtl = fview("xtl")
        xtr = fview("xtr")
        wqt = fview("wqt")
        bqp = fview("bqp")
        bkp = fview("bkp")
        wkt = fview("wkt")
        wvt = fview("wvt")
        bvr = fview("bvr")
        bprr = fview("bprr")
        ident = fview("ident")
        wpt = fview("wpt")
        wrt = fview("wrt")
        mgboth = fview("mgboth")
        xlb = fview("xlb")
        xrb = fview("xrb")
        aglr = fview("aglr")
        agrr = fview("agrr")
        obbl = fview("obbl")
        obb2 = fview("obb2")
        cent = fview("cent")
        brp = fview("brp")

        # ---- PE warm-up from memset tiles + ACT table preloads ----
        wsrc = cst.tile([128, 512], f16, tag="wsrc")
        nc.vector.memset(wsrc, 0.5)
        pw = warm_p.tile([128, 512], f32, tag="warm")
        for wi in range(6):
            nc.tensor.matmul(pw, wsrc[:, 0:128], wsrc,
                             start=True, stop=True, skip_group_check=True)
        wact = cst.tile([1, 32], f32, tag="wact")
        nc.vector.memset(wact, 1.0)
        nc.scalar.activation(out=wact, in_=wact, func=AF.Sqrt)
        nc.scalar.activation(out=wact, in_=wact, func=AF.Exp)

        wsrc32 = cst.tile([128, 512], f32, tag="wsrc32")
        nc.vector.memset(wsrc32, 0.25)

        def dmy(dep, n=1, cols=96):
            # keep-warm matmul pinned in time by a data dependency: the
            # scheduler cannot hoist it ahead of `dep` (SBUF tile).
            kp = dep.shape[0]
            lhs = dep if dep.shape[-1] <= 64 else dep[:, 0:64]
            mov = wsrc if dep.dtype == f16 else wsrc32
            for _ in range(n):
                nc.tensor.matmul(pw[0:64, 0:cols],
                                 lhs, mov[0:kp, 0:cols],
                                 start=True, stop=True, skip_group_check=True)

        # identh for the fp16 mask transpose (gpsimd, idle early)
        identh = cst.tile([64, 64], f16, tag="identh")
        nc.gpsimd.tensor_copy(identh, ident)

        eps_t = cst.tile([C, 1], f32, tag="eps")
        nc.vector.memset(eps_t, 1e-5)
        eps128 = cst.tile([128, 1], f32, tag="eps128")
        nc.vector.memset(eps128, 1e-5)

        # ---- attention: q.T, k.T ----
        qt = wk.tile([128, KT, C], f32, tag="qt")
        ktl = wk.tile([128, KT, C], f32, tag="ktl")
        for (src, w, bias, dst) in [(xtl, wqt, bqp, qt), (xtr, wkt, bkp, ktl)]:
            for ut in range(KT):
                p = ps.tile([128, C], f32, tag="ps")
                for kt in range(KT):
                    nc.tensor.matmul(p, w[:, kt, ut * 128:(ut + 1) * 128],
                                     src[:, kt], start=(kt == 0), stop=(kt == KT - 1))
                nc.vector.tensor_scalar(out=dst[:, ut], in0=p,
                                        scalar1=bias[:, ut:ut + 1],
                                        scalar2=None, op0=OP.add)

        # ---- proj psum bias pre-init (DVE idle during the energy MMs);
        # both sides share one PSUM tile so LN1 stats run fused ----
        ppb = proj_p.tile([C, 2, T], f32, tag="projb")
        nc.vector.tensor_copy(ppb, bprr)

        # ---- energy -> exp(energy/16), rowsum fused via accum_out ----
        # (|energy/16| < 0.75 so no row-max subtraction is needed)
        pe_ = ps.tile([C, C], f32, tag="ps")
        for ut in range(KT):
            nc.tensor.matmul(pe_, qt[:, ut], ktl[:, ut],
                             start=(ut == 0), stop=(ut == KT - 1))
        attn = wk.tile([C, C], f32, tag="attn")
        rowsum = sm.tile([C, 1], f32, tag="rowsum")
        nc.scalar.activation(out=attn, in_=pe_, func=AF.Exp, scale=1.0 / 16.0,
                             accum_out=rowsum)
        nc.vector.reciprocal(rowsum, rowsum)

        # ---- v = (x_l - x_r) @ Wv.T + bv  (fills the softmax PE window) ----
        xdt = wk.tile([128, KT, C], f32, tag="xdt")
        nc.vector.tensor_sub(xdt, xtl, xtr)
        pv = ps.tile([C, T], f32, tag="ps")
        for kt in range(KT):
            nc.tensor.matmul(pv, xdt[:, kt], wvt[:, kt],
                             start=(kt == 0), stop=(kt == KT - 1))
        v_sb = wk.tile([C, T], f32, tag="v")
        nc.vector.tensor_tensor(out=v_sb, in0=pv, in1=bvr, op=OP.add)
        nc.vector.tensor_scalar_mul(attn, attn, rowsum)

        # ---- attn.T ----
        pat = ps.tile([C, C], f32, tag="ps")
        nc.tensor.transpose(pat, attn, ident)
        attnT = wk.tile([C, C], f32, tag="attnT")
        nc.vector.tensor_copy(attnT, pat)

        # ---- out_l.T / out_r.T  [u, c] ----
        oLT = wk.tile([128, KT, C], f32, tag="oLT")
        oRT = wk.tile([128, KT, C], f32, tag="oRT")
        for ut in range(KT):
            pl = ps.tile([128, C], f32, tag="ps")
            nc.tensor.matmul(pl, v_sb[:, ut * 128:(ut + 1) * 128], attnT,
                             start=True, stop=True)
            nc.vector.tensor_copy(oLT[:, ut], pl)
            pr = ps.tile([128, C], f32, tag="ps")
            nc.tensor.matmul(pr, v_sb[:, ut * 128:(ut + 1) * 128], attn,
                             start=True, stop=True)
            nc.scalar.copy(oRT[:, ut], pr)

        # ---- proj (accumulates onto pre-loaded bias); side L's stats are
        # emitted between the two sides so they overlap proj-R ----
        st1 = sm.tile([C, 2, 6], f32, tag="st1")
        for ut in range(KT):
            nc.tensor.matmul(ppb[:, 0], oLT[:, ut], wpt[:, ut],
                             start=False, stop=(ut == KT - 1))
        nc.vector.bn_stats(out=st1[:, 0], in_=ppb[:, 0])
        for ut in range(KT):
            nc.tensor.matmul(ppb[:, 1], oRT[:, ut], wpt[:, ut],
                             start=False, stop=(ut == KT - 1))
        nc.vector.bn_stats(out=st1[:, 1], in_=ppb[:, 1])
        # dense PE bridge over the LN1 window (HAM stays at 8/8)
        dmy(oLT[:, 0], 5, 512)

        # ---- LN1: stats fused over the shared [C,2,T] PSUM tile; side L
        # is normalized (and transposed, below) before side R. ----
        OUT_L = wk.tile([C, T], f32, tag="OUTL")
        OUT_R = wk.tile([C, T], f32, tag="OUTR")
        mv1 = sm.tile([C, 2, 2], f32, tag="mv1")
        nc.vector.bn_aggr(out=mv1[:, 0], in_=st1[:, 0])
        nc.vector.bn_aggr(out=mv1[:, 1], in_=st1[:, 1])
        rstd1 = sm.tile([C, 2], f32, tag="rs1")
        nc.scalar.activation(out=rstd1, in_=mv1[:, :, 1:2], func=AF.Sqrt,
                             bias=eps_t)
        nc.vector.reciprocal(rstd1, rstd1)
        for s, gam, bres, outt in ((0, aglr, xlb, OUT_L),
                                   (1, agrr, xrb, OUT_R)):
            nc.vector.tensor_scalar(out=outt, in0=ppb[:, s],
                                    scalar1=mv1[:, s, 0:1],
                                    scalar2=rstd1[:, s:s + 1],
                                    op0=OP.subtract, op1=OP.mult)
            nc.vector.tensor_tensor(out=outt, in0=outt, in1=gam, op=OP.mult)
            nc.vector.tensor_tensor(out=outt, in0=outt, in1=bres, op=OP.add)

        # ---- transposes -> [u(128), kt, c] f32 (router) + f16 oAll (experts) ----
        oLT2 = wk.tile([128, KT, C], f32, tag="oLT2")
        oRT2 = wk.tile([128, KT, C], f32, tag="oRT2")
        oAll = wk.tile([128, KT, 2, C], f16, tag="oAll")
        for (side, src, dst) in [(0, OUT_L, oLT2), (1, OUT_R, oRT2)]:
            for ut in range(KT):
                pt = ps.tile([128, C], f32, tag="ps")
                nc.tensor.transpose(pt, src[:, ut * 128:(ut + 1) * 128], ident)
                nc.vector.tensor_copy(dst[:, ut], pt)
                nc.scalar.copy(oAll[:, ut, side], pt)

        # ---- router: xp.T then sims (no norms: top-k is scale-invariant
        #      per row and the top-2 softmax gate is exactly 1.0) ----
        rtiles = [(oLT2, 0), (oLT2, 1), (oRT2, 0), (oRT2, 1)]
        pxp = ps.tile([EXP, C], f32, tag="ps")
        for j, (tl, kt) in enumerate(rtiles):
            nc.tensor.matmul(pxp, wrt[:, j], tl[:, kt],
                             start=(j == 0), stop=(j == 3))
        xpT = sm.tile([EXP, C], f32, tag="xpT")
        nc.vector.tensor_scalar(out=xpT, in0=pxp, scalar1=brp, scalar2=None,
                                op0=OP.add)
        psim = ps.tile([C, C], f32, tag="ps")
        nc.tensor.matmul(psim, xpT, cent, start=True, stop=True)

        # top-8 straight off PSUM; full routing matrix via threshold:
        # R[c, e] = (sim >= 2nd-largest).  No exact ties on this input.
        mx8 = sm.tile([C, 8], f32, tag="mx8")
        nc.vector.max(out=mx8, in_=psim)
        maskt = sm.tile([C, C], f16, tag="maskt")
        nc.vector.tensor_scalar(out=maskt, in0=psim, scalar1=mx8[:, 1:2],
                                scalar2=None, op0=OP.is_ge)
        pmt = ps.tile([C, C], f16, tag="psh")
        nc.tensor.transpose(pmt, maskt, identh)
        RTeh = wk.tile([C, C], f16, tag="RTeh")
        nc.vector.tensor_copy(RTeh, pmt)
        # ALL 64 experts' replicated masks via DRAM round-trip on the
        # sync HWDGE queue -- empty by now; the input stream is done
        # (write + 4 pipelined broadcast reads)
        rtd = dram.tile([C, C], f16)
        nc.sync.dma_start(out=rtd[:], in_=RTeh)
        rsrc = rtd[:]
        mreps = []
        for rc in range(4):
            mt = wk.tile([128, 16, C], f16, tag=f"mrep{rc}")
            src_ap = bass.AP(tensor=rsrc.tensor,
                             offset=rsrc.offset + rc * 16 * C,
                             ap=[[0, 128], [C, 16], [1, C]])
            nc.sync.dma_start(out=mt, in_=src_ap)
            mreps.append(mt)
        # dense PE bridge over the mask round-trip window
        dmy(RTeh, 8, 512)

        # ---- expert stage (e0 starts the PSUM accumulation group) ----
        ps_moe = moe_p.tile([128, T], f32, tag="psmoe")

        def asch_mult(dst, msrc_ap):
            # dst[p, e(EG), kt, side, c] = oAll[p, kt, side, c] * m[e, c]
            out_ap = bass.AP(tensor=dst.tensor, offset=dst.offset,
                             ap=[list(dst.ap[0]), [KT * 2 * C, EG],
                                 [1, KT * 2 * C]])
            in0 = bass.AP(tensor=oAll.tensor, offset=oAll.offset,
                          ap=[list(oAll.ap[0]), [0, EG], [1, KT * 2 * C]])
            nc.vector.tensor_tensor(out=out_ap, in0=in0, in1=msrc_ap, op=OP.mult)

        def mask_bcast_ap(mt, col0):
            # [p, e(EG), kt*side(bcast), c] view of a [128, 16, C] tile
            return bass.AP(tensor=mt.tensor, offset=mt.offset + col0 * C,
                           ap=[list(mt.ap[0]), [C, EG], [0, KT * 2], [1, C]])

        NG = C // EG
        for g in range(NG):
            e0 = g * EG
            msrc = mask_bcast_ap(mreps[e0 // 16], e0 % 16)
            asch = asc_p.tile([128, EG, KT, 2, C], f16, tag="asc")
            asch_mult(asch, msrc)
            for i in range(EG):
                for kt in range(KT):
                    nc.tensor.matmul(
                        ps_moe, asch[:, i, kt], weq[:, e0 + i, kt],
                        start=(g == 0 and i == 0 and kt == 0),
                        stop=(g == NG - 1 and i == EG - 1
                              and kt == KT - 1),
                        skip_group_check=True)
            if g == 3:
                # bias (be * WE_SCALE) via R matmuls, off the LN2 path and
                # inside the k=8/8 zone
                nc.tensor.matmul(ps_moe[0:C], RTeh, behs, start=False,
                                 stop=False, skip_group_check=True)
                nc.tensor.matmul(ps_moe[C:128], RTeh, behs, start=False,
                                 stop=False, skip_group_check=True)

        # ---- beta+residual for LN2 on gpsimd during the expert phase;
        # the R side is partition-shifted to rows 64:128 by an on-chip
        # SBUF->SBUF DMA (engines need aligned start partitions) ----
        obboth = wk.tile([128, T], f32, tag="obboth")
        obr = wk.tile([C, T], f32, tag="obr")
        nc.gpsimd.tensor_add(obboth[0:C], OUT_L, obbl)
        nc.gpsimd.tensor_add(obr, OUT_R, obb2)
        nc.gpsimd.dma_start(out=obboth[C:128], in_=obr)

        # ---- final LN + residual: both sides live on disjoint partition
        # ranges of ps_moe, so ONE fused chain covers both ----
        obv = wk.tile([128, T], f32, tag="obv")
        st2 = sm.tile([128, 6], f32, tag="st2")
        nc.vector.bn_stats(out=st2, in_=ps_moe)
        mv2 = sm.tile([128, 2], f32, tag="mv2")
        nc.vector.bn_aggr(out=mv2, in_=st2)
        rstd2 = sm.tile([128, 1], f32, tag="rs2")
        nc.scalar.activation(out=rstd2, in_=mv2[:, 1:2], func=AF.Sqrt,
                             bias=eps128)
        nc.vector.reciprocal(rstd2, rstd2)
        nc.vector.tensor_scalar(out=obv, in0=ps_moe, scalar1=mv2[:, 0:1],
                                scalar2=rstd2, op0=OP.subtract, op1=OP.mult)
        nc.vector.tensor_tensor(out=obv, in0=obv, in1=mgboth, op=OP.mult)
        nc.vector.tensor_tensor(out=obv, in0=obv, in1=obboth, op=OP.add)
        nc.scalar.dma_start(out=ob_d.ap(), in_=obv)

    nc.compile()
    return nc


def _tile_t(w):
    # (T_in, N) -> [128, T_in//128, N] partition-tiled
    t_in, n = w.shape
    return np.ascontiguousarray(w.reshape(t_in // 128, 128, n).transpose(1, 0, 2))


def _prep_in_maps(inputs):
    import ml_dtypes

    f = np.float32
    x_l, x_r = inputs["x_l"], inputs["x_r"]

    def rep(v):
        return np.repeat(np.asarray(v, f).reshape(1, T), C, axis=0)

    cen = np.asarray(inputs["centers"], f)
    cenn = cen / np.maximum(np.linalg.norm(cen, axis=-1, keepdims=True), 1e-12)
    arrs = {
        "wqt": _tile_t(np.asarray(inputs["Wq"], f).T),
        "wkt": _tile_t(np.asarray(inputs["Wk"], f).T),
        "wvt": _tile_t(np.asarray(inputs["Wv"], f).T),
        "wpt": _tile_t(np.asarray(inputs["Wp"], f).T),
        "bqp": np.asarray(inputs["bq"], f).reshape(KT, 128).T,
        "bkp": np.asarray(inputs["bk"], f).reshape(KT, 128).T,
        "wrt": _tile_t(np.asarray(inputs["Wr"], f).T),
        "brp": np.asarray(inputs["br"], f).reshape(EXP, 1),
        "cent": np.ascontiguousarray(cenn.T),
        "ident": np.eye(64, dtype=f),
        "bvr": rep(inputs["bv"]),
        "bprr": np.concatenate([rep(inputs["bp"]), rep(inputs["bp"])], axis=1),
        "aglr": rep(inputs["ag_l"]), "agrr": rep(inputs["ag_r"]),
        "mgboth": np.concatenate([rep(inputs["mg_l"]), rep(inputs["mg_r"])]),
        "obbl": rep(inputs["mb_l"]), "obb2": rep(inputs["mb_r"]),
        "xlb": np.zeros((C, T), f), "xrb": np.zeros((C, T), f),
    }
    behs = (np.asarray(inputs["be"], f) * WE_SCALE).astype(np.float16)
    # We -> [128(t%128), C, KT, T(u)] fp8e4, x128 (layer_norm absorbs it)
    We = np.asarray(inputs["We"], f)
    WeT = We.transpose(0, 2, 1).reshape(C, KT, 128, T).transpose(2, 0, 1, 3)
    weq = np.ascontiguousarray(WeT * WE_SCALE).astype(ml_dtypes.float8_e4m3)

    def pack(cname, extra):
        parts = CHUNK_SPECS[cname][0]
        blob = np.zeros((parts, _chunk_cols(cname)), f)
        for name, shape in CHUNK_SPECS[cname][1]:
            _, off, _ = FIELD_OFF[name]
            cols = int(np.prod(shape[1:]))
            a = extra[name] if name in extra else arrs[name]
            blob[0:shape[0], off:off + cols] = \
                np.asarray(a, f).reshape(shape[0], cols)
        return blob

    static = {c: pack(c, {}) for c in
              ("cK", "cV", "cVb", "cP", "cR")}
    abl = np.asarray(inputs["ab_l"], f).reshape(1, T)
    abr = np.asarray(inputs["ab_r"], f).reshape(1, T)
    in_maps = []
    for b in range(N_CORES):
        xtl = _tile_t(np.ascontiguousarray(np.asarray(x_l[b], f).T))
        xtr = _tile_t(np.ascontiguousarray(np.asarray(x_r[b], f).T))
        cA = pack("cA", {"xtl": xtl, "xtr": xtr})
        cRb = pack("cRb", {"xlb": np.asarray(x_l[b], f) + abl,
                           "xrb": np.asarray(x_r[b], f) + abr})
        m = {"cA": cA, "cRb": cRb, "behs": behs, "weq": weq}
        m.update(static)
        in_maps.append(m)
    return in_maps


def kernel(**inputs) -> np.ndarray:
    from concourse.bass_utils import run_bass_kernel_spmd

    if "nc" not in _CACHE:
        _CACHE["nc"] = _build()
    nc = _CACHE["nc"]
    in_maps = _prep_in_maps(inputs)
    res = run_bass_kernel_spmd(nc, in_maps, list(range(N_CORES)))
    _CACHE["exec_time_ns"] = res.exec_time_ns
    both = np.stack([res.results[b]["oboth"] for b in range(N_CORES)])
    return np.stack([both[:, 0:C], both[:, C:128]]).astype(np.float32)
